# revision 34
# baseline (speedup 1.0000x reference)
"""Trainium2 Bass kernel for nn_ConstraintsModule (fuzzy-logic constraint
propagation).

Algorithm notes
---------------
The reference computes, twice (apply-1 with active=full_body, apply-2 with
active=unsat_head and goal-masked bodies):

    body_rev[b,c,a] = pb[c,a] + v[b,a]*(nb-pb)      -> max over a
    body_min[b,c]   = active[b,c] * (1 - max_a body_rev)
    lb[n] = max_c body_min * pos_head[c,n] ; ub = 1 - max_c body_min*neg_head
    u = max(min(lb,ub), min(max(lb,ub), v))

Bodies are sparse (~4 literals/constraint) and heads one-hot, so per
constraint we only gather its literal-value rows and min-reduce their
complements:  bm = 1 - max_a(v) = min_a(1 - v).

Key tricks:
1. The `active` gate folds into the gathered VALUES: a literal row whose
   goal-condition fails gets complement value -1, making bm <= 0, and
   relu() at the scatter stage reproduces active=0 exactly.  For apply-2
   (active = head literal unsatisfied by goal) one extra "head
   activation" row is appended per constraint.  This removes the
   goal@body equality matmul and its big operand loads completely.
2. Precision: apply-2 consumes BOTH u1 and 1-u1; a 16-bit u1 near 1 would
   lose all relative accuracy of 1-u1, so launch 1 runs f32 end-to-end
   (f32 tables, f32 reduces, f32 matmuls through the PE at 4 cyc/row).
   Launch 2 only needs u2 itself to be accurate, so it runs bf16 with
   head-sign-specific value spaces: pos-head slots store complements
   v'=1-v (bm selections), neg-head slots store negated originals -v
   (so ub = min_c W_c is a pure selection; empty scatter cells are
   neutralized by a static +2 bias).
3. Engine schedule: the shared HWDGE unit serializes descriptor prep
   (625ns per DMA) and the DMA engines are modeled as one serial
   resource, so blobs are split across the SP HWDGE and the Pool SWDGE
   ring in consumption order (launch 1: vah1/val/vb on SP, vah2/bs on
   Pool; launch 2: vah/vb on SP, val on Pool).  The register-move
   preamble is stripped (-250ns head) and dummy PE matmuls bridge the
   input wait so the real matmuls run at the full-clock p-state.
4. Launch 1's post-matmul reduction: the relu floors are absorbed by the
   median clamp u = min(max(p,lo),hi) = median(p,lb,ub) (any all-gated
   side produces an out-of-range bound that the median ignores), and the
   per-layer +1 of ub = min_l(1-ps_l) rides the scatter weight of a
   constant-one bm slot (CONST_SLOT, all-pad rows).  Both sides then
   reduce with ONE negated min-chain over paired [128, 2B] PSUM tiles
   (left = neg layer, right = pos layer): t_l = min(t_{l-1}, -P_l) via
   scalar_tensor_tensor (one PSUM input per op — walrus limit), with the
   first step on Act.  Launch 2 keeps per-side chains (relu/bias ts on
   Act+DVE) which measured faster there.

Sharding: constraints are owned by the core that owns their head atom
(atom range of 128 per core), so the head-scatter and clamp are core-local.
The host gathers per-literal value rows between launches (pure layout).
"""
import numpy as np

import concourse.bass as bass
import concourse.tile as tile
from concourse import mybir
from concourse.tile import ScopedClock
from concourse.bass_utils import run_bass_kernel_spmd

B = 128
NCOL = 2048
NA = 1024
C = 512
NCORES = 8
SLOTS = 128          # constraint slots per core (padded)
NLOC = 128           # atoms per core
# value-table regions (row ids):
REG_VP = 0 * NA      # complement-space pos-literal rows (both launches)
REG_VN = 1 * NA      # complement-space neg-literal rows
REG_WP = 2 * NA      # launch-2 neg-head slots: negated pos-literal rows
REG_WN = 3 * NA      # launch-2 neg-head slots: negated neg-literal rows
REG_HP = 4 * NA      # launch-2 pos-head slots: head-activation rows
REG_HN = 5 * NA      # launch-2 neg-head slots: head-activation rows
ZROW = 6 * NA        # neutral (+1) padding row
CONST_SLOT = SLOTS - 1  # all-pad slot: bm == 1, carries folded layer biases

WARM1 = 43           # PE p-state warm matmul counts (retuned from traces)
WARM2 = 36


class FixedTileContext(tile.TileContext):
    """Two workarounds for this walrus/NRT combo: (1) skip the tail
    clear_and_free_semaphores — its InstSemClear makes NRT reject the NEFF at
    load, and NRT resets semaphores per execution anyway; (2) multi-wait
    instructions are split afterwards by split_multi_waits()."""

    def _drain_and_barrier(self, tick_clock, wait_clock):
        drain_inst = self.nc.sync.drain()
        wait_clock.add_sem_waits(
            drain_inst.ins, ScopedClock({None: tick_clock.global_clock})
        )
        self.nc.all_engine_barrier()
        assert self.sems is not None
        popped = self.nc._tile_sem_poison_stack.pop()
        assert popped is self._sem_poison
        self.nc.all_engine_barrier()


def split_multi_waits(nc: bass.Bass) -> int:
    """walrus here accepts only ONE sync wait per instruction; Tile's
    add_semaphores attaches several.  Hoist all but one wait onto fresh
    same-engine nops placed immediately before the instruction (engine
    program order is preserved, so blocking semantics are identical)."""
    n_split = 0
    for f in nc.m.functions:
        for b in f.blocks:
            new = []
            for ins in b.instructions:
                si = ins.sync_info
                waits = list(si.on_wait) if si and si.on_wait else []
                if len(waits) > 1:
                    for w in waits[:-1]:
                        nop = mybir.InstNoOp(
                            name=f"waitsplit-{n_split}", ins=[], outs=[])
                        n_split += 1
                        nop.engine = ins.engine
                        nop.sync_info = mybir.SyncInfo(on_wait=[w], on_update=[])
                        new.append(nop)
                    ins.sync_info = mybir.SyncInfo(
                        on_wait=[waits[-1]],
                        on_update=list(si.on_update) if si.on_update else [])
                new.append(ins)
            b.instructions = new
    return n_split


_PROGRAM_CACHE = {}
SPLIT_WAITS = True  # set False when running under CoreSim / TimelineSim


def strip_preamble(nc: bass.Bass):
    """Remove the const-AP memsets and the initial all-engine barrier from
    the entry block.  Valid because (a) NRT resets semaphores per execution,
    (b) no instruction reads the const APs (activation biases come from our
    own DMA'd blobs)."""
    main = nc.m.functions[0].blocks[0]
    main.instructions = [
        ins for ins in main.instructions
        if not isinstance(ins, (mybir.InstMemset, mybir.InstDrain,
                                mybir.InstEventSemaphore,
                                mybir.InstRegisterMove))
    ]


def strip_epilogue(nc: bass.Bass):
    """Keep only the first drain of the end block (it carries the global
    tile-clock sem waits, incl. the output-DMA completion) and drop the two
    all-engine barrier rounds behind it."""
    for blk in nc.m.functions[0].blocks:
        if not blk.name.endswith("_end"):
            continue
        kept, seen_drain = [], False
        for ins in blk.instructions:
            if isinstance(ins, mybir.InstDrain):
                if not seen_drain:
                    kept.append(ins)
                    seen_drain = True
                continue
            if isinstance(ins, mybir.InstEventSemaphore):
                continue
            kept.append(ins)
        blk.instructions = kept


def _col_min_tree(eng, pool, src, k, out_ap, name, dt):
    """Min over the k columns of src ([64, k, B] AP, base partition 0),
    written into out_ap ([64, B], any base partition).  Uses bulk
    first-half-vs-second-half tensor_tensor ops (equal input base
    partitions — required by the BIR verifier)."""
    mn = mybir.AluOpType.min
    cur, i = src, 0
    while k > 3:
        assert k % 2 == 0, f"host must pad col count even, got {k}"
        h = k // 2
        t = pool.tile([64, h, B], dt, tag=f"{name}t{i}")
        eng.tensor_tensor(t[:], cur[:, 0:h, :], cur[:, h:2 * h, :], mn)
        cur, k, i = t[:], h, i + 1
    if k == 3:
        t = pool.tile([64, B], dt, tag=f"{name}p")
        eng.tensor_tensor(t[:], cur[:, 0, :], cur[:, 1, :], mn)
        eng.tensor_tensor(out_ap, t[:], cur[:, 2, :], mn)
    elif k == 2:
        eng.tensor_tensor(out_ap, cur[:, 0, :], cur[:, 1, :], mn)
    else:
        eng.tensor_tensor(out_ap, cur[:, 0, :], cur[:, 0, :], mn)


def _warm_pe(nc, pool, psum, n):
    """Dependency-free dummy matmuls on a scratch tile keep the PE busy
    through the input-DMA wait, so it reaches the full-clock p-state
    (3us of continuous execution) and is still running when the real
    matmuls issue (a gap would reset the ramp).  The memset runs on DVE
    (idle until the first value blob lands)."""
    scr = pool.tile([128, B], mybir.dt.bfloat16, tag="warm_in")
    nc.vector.memset(scr[:], 0.0)
    pscr = psum.tile([128, B], mybir.dt.float32, tag="warm_out")
    for _ in range(n):
        nc.tensor.matmul(pscr[:], scr[:], scr[:], start=True, stop=True)


def _build_p1(KH: int, KL: int, lpos: int, lneg: int) -> bass.Bass:
    """Launch-1 program: f32 end-to-end (u1 and 1-u1 both must stay
    relatively accurate for apply-2's tables)."""
    key = ("p1", KH, KL, lpos, lneg)
    if key in _PROGRAM_CACHE:
        return _PROGRAM_CACHE[key]

    f32 = mybir.dt.float32
    mx, mn = mybir.AluOpType.max, mybir.AluOpType.min
    L = lpos + lneg
    HH = KH // 2 if (KH > 3 and KH % 2 == 0) else KH
    LPAD = 2 * max(lpos, lneg)
    VBW = LPAD * NLOC                    # scat layers (pos | neg, padded)
    nc = bass.Bass(num_devices=NCORES)
    vah1_d = nc.declare_dram_parameter("vah1", [64, HH * B], f32, isOutput=False)
    vah2_d = nc.declare_dram_parameter("vah2", [64, HH * B], f32, isOutput=False)
    val_d = nc.declare_dram_parameter("val", [64, KL * B], f32, isOutput=False)
    vb_d = nc.declare_dram_parameter("vb", [128, VBW], f32, isOutput=False)
    bs_d = nc.declare_dram_parameter("bs", [NLOC, B], f32, isOutput=False)
    u_d = nc.declare_dram_parameter("u", [NLOC, B], f32, isOutput=True)

    with FixedTileContext(nc) as tc:
        with (
            tc.tile_pool(name="sbuf", bufs=1) as pool,
            tc.tile_pool(name="psum", bufs=1, space="PSUM") as psum,
        ):
            _warm_pe(nc, pool, psum, WARM1)
            # DMA plan (consumption/engine order; HWDGE prep is 625ns
            # serialized, the DMA engines pick ready transfers in order):
            #   SP HWDGE : vah1 (first hi half), val (small), vb
            #   Pool SWDGE: vah2 (second hi half), bs (clamp base, last)
            vah1 = pool.tile([64, B, HH], f32)
            nc.sync.dma_start(vah1[:],
                              vah1_d[:].rearrange("p (b k) -> p b k", k=HH))
            vah2 = pool.tile([64, B, HH], f32)
            nc.gpsimd.dma_start(vah2[:],
                                vah2_d[:].rearrange("p (b k) -> p b k", k=HH))
            val = pool.tile([64, KL, B], f32)
            nc.sync.dma_start(val[:],
                              val_d[:].rearrange("p (k b) -> p k b", k=KL))
            vb = pool.tile([128, VBW], f32)
            nc.sync.dma_start(vb[:], vb_d[:])
            bs = pool.tile([NLOC, B], f32)
            nc.gpsimd.dma_start(bs[:], bs_d[:])

            # bm: one bulk tensor_reduce per hi half, min-combined in place;
            # the lo (val) tree is one tt for KL<=2.
            ra = pool.tile([64, B], f32)
            nc.vector.tensor_reduce(out=ra[:], in_=vah1[:],
                                    axis=mybir.AxisListType.X, op=mn)
            bm = pool.tile([128, B], f32)
            nc.vector.tensor_reduce(out=bm[0:64, :], in_=vah2[:],
                                    axis=mybir.AxisListType.X, op=mn)
            nc.vector.tensor_tensor(bm[0:64, :], ra[:], bm[0:64, :], mn)
            _col_min_tree(nc.vector, pool, val[:], KL, bm[64:128, :], "l", f32)

            # f32 matmuls (4 cyc/row) into PAIRED PSUM tiles: left half =
            # neg layer l, right half = pos layer l.  The per-layer +1 (for
            # ub = min_l(1-ps)) rides the const-one bm slot's weight, and the
            # relu floors are absorbed by the median clamp, so the whole
            # post-matmul reduction is one negated min-chain:
            #   t_l = min(t_{l-1}, -P_l)  ->  t = [ub | -lb]
            LP = max(lpos, lneg)
            ps = []
            for l in range(LP):
                pt = psum.tile([NLOC, 2 * B], f32, tag=f"ps{l}")
                nc.tensor.matmul(pt[:, 0:B], vb[:, (LP + l) * NLOC:
                                                 (LP + l + 1) * NLOC],
                                 bm[:], start=True, stop=True)
                nc.tensor.matmul(pt[:, B:2 * B], vb[:, l * NLOC:
                                                    (l + 1) * NLOC],
                                 bm[:], start=True, stop=True)
                ps.append(pt)

            t0 = pool.tile([NLOC, 2 * B], f32, tag="t0")
            nc.vector.tensor_scalar(
                t0[:], ps[0][:], -1.0, None, mybir.AluOpType.mult)
            acc = t0
            for l in range(1, LP):
                nxt = pool.tile([NLOC, 2 * B], f32, tag=f"t{l}")
                nc.vector.scalar_tensor_tensor(
                    nxt[:], ps[l][:], -1.0, acc[:], mybir.AluOpType.mult, mn)
                acc = nxt

            # acc = [ub | -lb]; u = clamp(p, lo, hi) = median(p, lb, ub)
            ubh = acc[:, 0:B]
            nlb = acc[:, B:2 * B]
            lo = pool.tile([NLOC, B], f32)
            nc.vector.scalar_tensor_tensor(
                lo[:], nlb, -1.0, ubh, mybir.AluOpType.mult, mn)
            hi = pool.tile([NLOC, B], f32)
            nc.vector.scalar_tensor_tensor(
                hi[:], nlb, -1.0, ubh, mybir.AluOpType.mult, mx)
            m1 = pool.tile([NLOC, B], f32)
            nc.vector.tensor_tensor(m1[:], bs[:], lo[:], mx)
            u = pool.tile([NLOC, B], f32)
            nc.vector.tensor_tensor(u[:], m1[:], hi[:], mn)
            nc.sync.dma_start(u_d[:], u[:])

    strip_preamble(nc)
    strip_epilogue(nc)
    if SPLIT_WAITS:
        split_multi_waits(nc)
    _PROGRAM_CACHE[key] = nc
    return nc


def _build_p2(KH: int, KL: int, lpos: int, lneg: int) -> bass.Bass:
    """Launch-2 program: bf16, head-sign-specific value spaces (baseline
    structure; changed vs baseline: vb rides the Pool SWDGE ring so the
    matmuls are not gated on its late HWDGE slot, val goes second on SP,
    and the warm count is tuned so the real matmuls hit the full-clock
    p-state with no idle gap)."""
    key = ("p2", KH, KL, lpos, lneg)
    if key in _PROGRAM_CACHE:
        return _PROGRAM_CACHE[key]

    f32, bf16 = mybir.dt.float32, mybir.dt.bfloat16
    mx, mn = mybir.AluOpType.max, mybir.AluOpType.min
    L = lpos + lneg
    VBW = (L + 1) * NLOC + 1             # scat layers | base | zero col
    nc = bass.Bass(num_devices=NCORES)
    HH = KH // 2 if (KH > 3 and KH % 2 == 0) else KH
    vah_d = nc.declare_dram_parameter("vah", [64, HH * B], bf16, isOutput=False)
    vah2_d = None
    if HH != KH:
        vah2_d = nc.declare_dram_parameter("vah2", [64, HH * B], bf16,
                                           isOutput=False)
    val_d = nc.declare_dram_parameter("val", [64, KL * B], bf16, isOutput=False)
    vb_d = nc.declare_dram_parameter("vb", [128, VBW], bf16, isOutput=False)
    u_d = nc.declare_dram_parameter("u", [NLOC, B], bf16, isOutput=True)

    with FixedTileContext(nc) as tc:
        with (
            tc.tile_pool(name="sbuf", bufs=1) as pool,
            tc.tile_pool(name="psum", bufs=1, space="PSUM") as psum,
        ):
            _warm_pe(nc, pool, psum, WARM2)
            vah = pool.tile([64, HH, B], bf16)
            nc.sync.dma_start(vah[:],
                              vah_d[:].rearrange("p (k b) -> p k b", k=HH))
            if vah2_d is not None:
                vah2 = pool.tile([64, HH, B], bf16)
                nc.sync.dma_start(vah2[:],
                                  vah2_d[:].rearrange("p (k b) -> p k b", k=HH))
            val = pool.tile([64, KL, B], bf16)
            nc.gpsimd.dma_start(val[:],
                               val_d[:].rearrange("p (k b) -> p k b", k=KL))
            vb = pool.tile([128, VBW], bf16)
            nc.sync.dma_start(vb[:], vb_d[:])

            bm = pool.tile([128, B], bf16)
            if vah2_d is not None:
                # per-half trees pipeline with the two blob arrivals
                h1 = pool.tile([64, B], bf16, tag="h1")
                _col_min_tree(nc.vector, pool, vah[:], HH, h1[:], "h", bf16)
                h2 = pool.tile([64, B], bf16, tag="h2")
                _col_min_tree(nc.vector, pool, vah2[:], HH, h2[:], "g", bf16)
                nc.vector.tensor_tensor(bm[0:64, :], h1[:], h2[:],
                                        mybir.AluOpType.min)
            else:
                _col_min_tree(nc.vector, pool, vah[:], HH, bm[0:64, :],
                              "h", bf16)
            _col_min_tree(nc.vector, pool, val[:], KL, bm[64:128, :], "l", bf16)

            # First neg+pos layers land in ONE paired PSUM tile so a single
            # wide Act copy-neg seeds BOTH sides' chains; later layers stay
            # single tiles feeding unpaired stt chains (both sides in
            # negated space: relu is median-absorbed, the neg-layer bias
            # rides the const-one slot's weights).
            pt0 = psum.tile([NLOC, 2 * B], f32, tag="p0")
            nc.tensor.matmul(pt0[:, 0:B], vb[:, lpos * NLOC:
                                              (lpos + 1) * NLOC],
                             bm[:], start=True, stop=True)
            nc.tensor.matmul(pt0[:, B:2 * B], vb[:, 0:NLOC],
                             bm[:], start=True, stop=True)
            ps = [None] * L
            order = []
            for l in range(1, max(lpos, lneg)):
                if l < lneg:
                    order.append(lpos + l)
                if l < lpos:
                    order.append(l)
            for l in order:
                pt = psum.tile([NLOC, B], f32, tag=f"ps{l}")
                nc.tensor.matmul(pt[:], vb[:, l * NLOC:(l + 1) * NLOC],
                                 bm[:], start=True, stop=True)
                ps[l] = pt

            t0 = pool.tile([NLOC, 2 * B], bf16, tag="t0")
            nc.scalar.activation(
                t0[:], pt0[:], mybir.ActivationFunctionType.Copy,
                bias=0.0, scale=-1.0)

            # ub = min_l(-ps_neg_l); nlb = min_l(-ps_pos_l) = -lb
            ub = t0[:, 0:B]
            for l in range(1, lneg):
                nxt = pool.tile([NLOC, B], bf16, tag=f"ub{l}")
                nc.vector.scalar_tensor_tensor(
                    nxt[:], ps[lpos + l][:], -1.0, ub,
                    mybir.AluOpType.mult, mn)
                ub = nxt[:]
            nlb = t0[:, B:2 * B]
            for l in range(1, lpos):
                nxt = pool.tile([NLOC, B], bf16, tag=f"lb{l}")
                nc.vector.scalar_tensor_tensor(
                    nxt[:], ps[l][:], -1.0, nlb,
                    mybir.AluOpType.mult, mn)
                nlb = nxt[:]

            lo = pool.tile([NLOC, B], bf16)
            nc.vector.scalar_tensor_tensor(
                lo[:], nlb, -1.0, ub, mybir.AluOpType.mult, mn)
            hi = pool.tile([NLOC, B], bf16)
            nc.vector.scalar_tensor_tensor(
                hi[:], nlb, -1.0, ub, mybir.AluOpType.mult, mx)

            # u = clamp(base, lo, hi) = min(max(base, lo), hi)
            base = vb[:, L * NLOC:(L + 1) * NLOC]
            m1 = pool.tile([NLOC, B], bf16)
            nc.vector.tensor_tensor(m1[:], base, lo[:], mx)
            u = pool.tile([NLOC, B], bf16)
            nc.vector.tensor_tensor(u[:], m1[:], hi[:], mn)
            nc.sync.dma_start(u_d[:], u[:])

    strip_preamble(nc)
    strip_epilogue(nc)
    if SPLIT_WAITS:
        split_multi_waits(nc)
    _PROGRAM_CACHE[key] = nc
    return nc


def _pad_k(k):
    """Smallest col count >= k that the bulk halving tree accepts
    (k = m * 2^j with m in {1,2,3})."""
    if k <= 3:
        return k
    c = 4
    while True:
        for m in (4, 6):
            if m * c // 4 >= k:
                return m * c // 4
        c *= 2


def _pack_rows(row_lists_core, KH, KL):
    """hi slots (0:64) -> rows_hi[s] cols 0..KH; lo slots (64:128) ->
    rows_lo[s-64] cols 0..KL; ZROW pads."""
    rows_hi = np.full((64, KH), ZROW, dtype=np.int64)
    rows_lo = np.full((64, KL), ZROW, dtype=np.int64)
    for s, rr in enumerate(row_lists_core):
        if s < 64:
            rows_hi[s, :len(rr)] = rr
        else:
            rows_lo[s - 64, :len(rr)] = rr
    return rows_hi, rows_lo


class _Prep:
    """Host-side, launch-independent preprocessing (slot assignment, literal
    row ids, scatter one-hots)."""

    def __init__(self, preds, goal, atoms, pos_body, neg_body, pos_head, neg_head):
        f32 = np.float32
        import ml_dtypes
        self.bf16 = ml_dtypes.bfloat16
        self.atoms = np.asarray(atoms)
        self.p = preds[:, self.atoms].astype(f32)            # [B, NA]
        self.g = goal[:, self.atoms].astype(f32)
        self.pT = np.ascontiguousarray(self.p.T)             # [NA, B]
        self.gT = np.ascontiguousarray(self.g.T)

        hsum = pos_head + neg_head
        assert np.all(hsum.sum(axis=1) == 1.0), "heads must be one-hot"
        self.h = np.argmax(hsum, axis=1)                     # [C]
        self.head_is_pos = pos_head[np.arange(C), self.h] == 1.0
        owner = self.h // NLOC

        pos_lists = [np.nonzero(pos_body[c])[0] for c in range(C)]
        neg_lists = [np.nonzero(neg_body[c])[0] for c in range(C)]
        # launch-1 rows: complement space for every slot, no head rows
        row1 = [np.concatenate([REG_VP + pos_lists[c], REG_VN + neg_lists[c]])
                .astype(np.int64) for c in range(C)]
        # launch-2 rows: sign-specific space + head-activation row
        row2 = []
        for c in range(C):
            if self.head_is_pos[c]:
                rp, rn, rh = REG_VP, REG_VN, REG_HP
            else:
                rp, rn, rh = REG_WP, REG_WN, REG_HN
            row2.append(np.concatenate([
                rp + pos_lists[c], rn + neg_lists[c],
                [rh + self.h[c]]]).astype(np.int64))
        ncnt = np.array([len(r) for r in row2])

        self.cons = []
        k1h = k1l = k2h = k2l = 1
        for i in range(NCORES):
            ci = np.nonzero(owner == i)[0]
            assert len(ci) <= SLOTS, f"core {i} has {len(ci)} constraints"
            ci = ci[np.argsort(-ncnt[ci], kind="stable")]
            self.cons.append(ci)
            c2 = ncnt[ci]
            k2h = max(k2h, int(c2[:64].max(initial=0)))
            k2l = max(k2l, int(c2[64:].max(initial=0)))
            c1 = c2 - 1                                      # no head row
            k1h = max(k1h, int(c1[:64].max(initial=0)))
            k1l = max(k1l, int(c1[64:].max(initial=0)))
        self.key1 = (_pad_k(k1h), _pad_k(k1l))
        self.key2 = (_pad_k(k2h), _pad_k(k2l))

        self.rows1 = []
        self.rows2 = []
        lpos_need = lneg_need = 1
        layer_asn = []
        for i in range(NCORES):
            ci = self.cons[i]
            self.rows1.append(_pack_rows([row1[c] for c in ci], *self.key1))
            self.rows2.append(_pack_rows([row2[c] for c in ci], *self.key2))

            counts = {}
            asn = []
            for s, c in enumerate(ci):
                key = (self.h[c] % NLOC, bool(self.head_is_pos[c]))
                l = counts.get(key, 0)
                counts[key] = l + 1
                asn.append((s, key[1], l, key[0]))
                if key[1]:
                    lpos_need = max(lpos_need, l + 1)
                else:
                    lneg_need = max(lneg_need, l + 1)
            layer_asn.append(asn)

        self.lpos, self.lneg = lpos_need, lneg_need
        LP = max(self.lpos, self.lneg)
        self.LP = LP
        # Scatter one-hot layers, [pos 0..LP | neg 0..LP].  Slot CONST_SLOT
        # (guaranteed free: <=127 constraints per core) has bm == 1 (all-pad
        # rows); its weight carries the per-layer bias: -1 on launch-1 neg
        # layers (ub = min_l(1-ps)), -(0|2) empty-cell bias on launch-2 neg
        # layers.  Relu floors are absorbed by the median clamp, so pos
        # layers need no bias at all.
        L = self.lpos + self.lneg
        self.scat1 = []       # p1: [128, 2*LP*NLOC] f32, const-slot biases
        self.scat2 = []       # p2: [128, L*NLOC] one-hots + const-slot bias
        for i in range(NCORES):
            assert len(self.cons[i]) <= SLOTS - 1, "need a free const slot"
            sc1 = np.zeros((SLOTS, 2 * LP, NLOC), dtype=f32)
            sc2 = np.zeros((SLOTS, L, NLOC), dtype=f32)
            occ = np.zeros((max(self.lneg, 1), NLOC), dtype=bool)
            for s, is_pos, l, n in layer_asn[i]:
                sc1[s, l if is_pos else LP + l, n] = 1.0
                sc2[s, l if is_pos else self.lpos + l, n] = 1.0
                if not is_pos:
                    occ[l, n] = True
            sc1[CONST_SLOT, LP:, :] = -1.0
            for l in range(self.lneg):
                sc2[CONST_SLOT, self.lpos + l, :] = np.where(
                    occ[l], 0.0, -2.0)
            self.scat1.append(np.ascontiguousarray(
                sc1.reshape(SLOTS, 2 * LP * NLOC)))
            self.scat2.append(np.ascontiguousarray(
                sc2.reshape(SLOTS, L * NLOC)).astype(self.bf16))

    def gather1(self, vcat):
        """Launch-1 f32 blobs: hi halves b-major [64, HH*B], val k-major."""
        KH, KL = self.key1
        HH = KH // 2 if (KH > 3 and KH % 2 == 0) else KH
        f32 = np.float32
        out = []
        for i in range(NCORES):
            rh, rl = self.rows1[i]
            out.append({
                "vah1": np.ascontiguousarray(
                    vcat[rh[:, :HH]].transpose(0, 2, 1)
                    .reshape(64, HH * B)).astype(f32),
                "vah2": np.ascontiguousarray(
                    vcat[rh[:, HH:2 * HH]].transpose(0, 2, 1)
                    .reshape(64, HH * B)).astype(f32),
                "val": np.ascontiguousarray(
                    vcat[rl].reshape(64, KL * B)).astype(f32),
            })
        return out

    def gather2(self, vcat):
        """Launch-2 bf16 blobs: vah k-major in two halves, val k-major."""
        KH, KL = self.key2
        HH = KH // 2 if (KH > 3 and KH % 2 == 0) else KH
        out = []
        for i in range(NCORES):
            rh, rl = self.rows2[i]
            m = {
                "vah": np.ascontiguousarray(
                    vcat[rh[:, :HH]].reshape(64, HH * B)).astype(self.bf16),
                "val": np.ascontiguousarray(
                    vcat[rl].reshape(64, KL * B)).astype(self.bf16),
            }
            if HH != KH:
                m["vah2"] = np.ascontiguousarray(
                    vcat[rh[:, HH:2 * HH]].reshape(64, HH * B)
                ).astype(self.bf16)
            out.append(m)
        return out


def kernel(preds, goal, atoms, pos_body, neg_body, pos_head, neg_head):
    preds = np.asarray(preds)
    f32 = np.float32
    prep = _Prep(np.asarray(preds, f32), np.asarray(goal, f32),
                 atoms, np.asarray(pos_body, f32),
                 np.asarray(neg_body, f32),
                 np.asarray(pos_head, f32),
                 np.asarray(neg_head, f32))
    nc1 = _build_p1(*prep.key1, prep.lpos, prep.lneg)
    nc2 = _build_p2(*prep.key2, prep.lpos, prep.lneg)
    core_ids = list(range(NCORES))
    g, p = prep.gT, prep.pT

    # ---- launch 1 (f32): complement-space table, goal-folded actives ----
    vcat1 = np.concatenate([
        np.where(g == 1.0, p, -1.0),             # v'+ = 1 - (1-p), sat-gated
        np.where(g == 0.0, 1.0 - p, -1.0),       # v'- = 1 - p-lit
        np.ones((4 * NA + 1, B), f32),           # unused regions + pad row
    ], axis=0)
    vas = prep.gather1(vcat1)
    in_maps = [dict(vas[i], vb=prep.scat1[i],
                    bs=np.ascontiguousarray(p[i * NLOC:(i + 1) * NLOC]))
               for i in range(NCORES)]
    res = run_bass_kernel_spmd(nc1, in_maps, core_ids)
    u1T = np.concatenate(
        [res.results[i]["u"].astype(f32) for i in range(NCORES)], axis=0)

    # ---- launch 2 (bf16): sign-specific spaces + head-activation rows ----
    a_ = (1.0 - g) * (1.0 - u1T)
    b_ = g * u1T
    vcat2 = np.concatenate([
        1.0 - a_,                                # v'+   (pos-head slots)
        1.0 - b_,                                # v'-
        -a_,                                     # -v+   (neg-head slots)
        -b_,                                     # -v-
        1.0 - 2.0 * g,                           # head-act, pos slots
        2.0 * g - 2.0,                           # head-act, neg slots
        np.ones((1, B), f32),                    # pad row
    ], axis=0)
    vas = prep.gather2(vcat2)
    zcol = np.zeros((128, 1), prep.bf16)
    in_maps = []
    for i in range(NCORES):
        vb = np.concatenate([
            prep.scat2[i],
            np.ascontiguousarray(
                u1T[i * NLOC:(i + 1) * NLOC]).astype(prep.bf16),
            zcol,
        ], axis=1)
        in_maps.append(dict(vas[i], vb=vb))
    res = run_bass_kernel_spmd(nc2, in_maps, core_ids)
    u2T = np.concatenate(
        [res.results[i]["u"].astype(f32) for i in range(NCORES)], axis=0)

    out = np.array(preds, dtype=preds.dtype, copy=True)
    out[:, prep.atoms] = u2T.T.astype(preds.dtype)
    return out


# revision 35
# speedup vs baseline: 1.0158x; 1.0158x over previous
"""Trainium2 Bass kernel for nn_ConstraintsModule (fuzzy-logic constraint
propagation).

Algorithm notes
---------------
The reference computes, twice (apply-1 with active=full_body, apply-2 with
active=unsat_head and goal-masked bodies):

    body_rev[b,c,a] = pb[c,a] + v[b,a]*(nb-pb)      -> max over a
    body_min[b,c]   = active[b,c] * (1 - max_a body_rev)
    lb[n] = max_c body_min * pos_head[c,n] ; ub = 1 - max_c body_min*neg_head
    u = max(min(lb,ub), min(max(lb,ub), v))

Bodies are sparse (~4 literals/constraint) and heads one-hot, so per
constraint we only gather its literal-value rows and min-reduce their
complements:  bm = 1 - max_a(v) = min_a(1 - v).

Key tricks:
1. The `active` gate folds into the gathered VALUES: a literal row whose
   goal-condition fails gets complement value -1, making bm <= 0, and
   relu() at the scatter stage reproduces active=0 exactly.  For apply-2
   (active = head literal unsatisfied by goal) one extra "head
   activation" row is appended per constraint.  This removes the
   goal@body equality matmul and its big operand loads completely.
2. Precision: apply-2 consumes BOTH u1 and 1-u1; a 16-bit u1 near 1 would
   lose all relative accuracy of 1-u1, so launch 1 runs f32 end-to-end
   (f32 tables, f32 reduces, f32 matmuls through the PE at 4 cyc/row).
   Launch 2 only needs u2 itself to be accurate, so it runs bf16 with
   head-sign-specific value spaces: pos-head slots store complements
   v'=1-v (bm selections), neg-head slots store negated originals -v
   (so ub = min_c W_c is a pure selection; empty scatter cells are
   neutralized by a static +2 bias).
3. Engine schedule: the shared HWDGE unit serializes descriptor prep
   (625ns per DMA) and the DMA engines are modeled as one serial
   resource, so blobs are split across the SP HWDGE and the Pool SWDGE
   ring in consumption order (launch 1: vah1/val/vb on SP, vah2/bs on
   Pool; launch 2: vah/vb on SP, val on Pool).  The register-move
   preamble is stripped (-250ns head) and dummy PE matmuls bridge the
   input wait so the real matmuls run at the full-clock p-state.
4. Launch 1's post-matmul reduction: the relu floors are absorbed by the
   median clamp u = min(max(p,lo),hi) = median(p,lb,ub) (any all-gated
   side produces an out-of-range bound that the median ignores), and the
   per-layer +1 of ub = min_l(1-ps_l) rides the scatter weight of a
   constant-one bm slot (CONST_SLOT, all-pad rows).  Both sides then
   reduce with ONE negated min-chain over paired [128, 2B] PSUM tiles
   (left = neg layer, right = pos layer): t_l = min(t_{l-1}, -P_l) via
   scalar_tensor_tensor (one PSUM input per op — walrus limit), with the
   first step on Act.  Launch 2 keeps per-side chains (relu/bias ts on
   Act+DVE) which measured faster there.

Sharding: constraints are owned by the core that owns their head atom
(atom range of 128 per core), so the head-scatter and clamp are core-local.
The host gathers per-literal value rows between launches (pure layout).
"""
import numpy as np

import concourse.bass as bass
import concourse.tile as tile
from concourse import mybir
from concourse.tile import ScopedClock
from concourse.bass_utils import run_bass_kernel_spmd

B = 128
NCOL = 2048
NA = 1024
C = 512
NCORES = 8
SLOTS = 128          # constraint slots per core (padded)
NLOC = 128           # atoms per core
# value-table regions (row ids):
REG_VP = 0 * NA      # complement-space pos-literal rows (both launches)
REG_VN = 1 * NA      # complement-space neg-literal rows
REG_WP = 2 * NA      # launch-2 neg-head slots: negated pos-literal rows
REG_WN = 3 * NA      # launch-2 neg-head slots: negated neg-literal rows
REG_HP = 4 * NA      # launch-2 pos-head slots: head-activation rows
REG_HN = 5 * NA      # launch-2 neg-head slots: head-activation rows
ZROW = 6 * NA        # neutral (+1) padding row
CONST_SLOT = SLOTS - 1  # all-pad slot: bm == 1, carries folded layer biases

WARM1 = 43           # PE p-state warm matmul counts (retuned from traces)
WARM2 = 36


class FixedTileContext(tile.TileContext):
    """Two workarounds for this walrus/NRT combo: (1) skip the tail
    clear_and_free_semaphores — its InstSemClear makes NRT reject the NEFF at
    load, and NRT resets semaphores per execution anyway; (2) multi-wait
    instructions are split afterwards by split_multi_waits()."""

    def _drain_and_barrier(self, tick_clock, wait_clock):
        drain_inst = self.nc.sync.drain()
        wait_clock.add_sem_waits(
            drain_inst.ins, ScopedClock({None: tick_clock.global_clock})
        )
        self.nc.all_engine_barrier()
        assert self.sems is not None
        popped = self.nc._tile_sem_poison_stack.pop()
        assert popped is self._sem_poison
        self.nc.all_engine_barrier()


def split_multi_waits(nc: bass.Bass) -> int:
    """walrus here accepts only ONE sync wait per instruction; Tile's
    add_semaphores attaches several.  Hoist all but one wait onto fresh
    same-engine nops placed immediately before the instruction (engine
    program order is preserved, so blocking semantics are identical)."""
    n_split = 0
    for f in nc.m.functions:
        for b in f.blocks:
            new = []
            for ins in b.instructions:
                si = ins.sync_info
                waits = list(si.on_wait) if si and si.on_wait else []
                if len(waits) > 1:
                    for w in waits[:-1]:
                        nop = mybir.InstNoOp(
                            name=f"waitsplit-{n_split}", ins=[], outs=[])
                        n_split += 1
                        nop.engine = ins.engine
                        nop.sync_info = mybir.SyncInfo(on_wait=[w], on_update=[])
                        new.append(nop)
                    ins.sync_info = mybir.SyncInfo(
                        on_wait=[waits[-1]],
                        on_update=list(si.on_update) if si.on_update else [])
                new.append(ins)
            b.instructions = new
    return n_split


_PROGRAM_CACHE = {}
SPLIT_WAITS = True  # set False when running under CoreSim / TimelineSim


def strip_preamble(nc: bass.Bass):
    """Remove the const-AP memsets and the initial all-engine barrier from
    the entry block.  Valid because (a) NRT resets semaphores per execution,
    (b) no instruction reads the const APs (activation biases come from our
    own DMA'd blobs)."""
    main = nc.m.functions[0].blocks[0]
    main.instructions = [
        ins for ins in main.instructions
        if not isinstance(ins, (mybir.InstMemset, mybir.InstDrain,
                                mybir.InstEventSemaphore,
                                mybir.InstRegisterMove))
    ]


def strip_epilogue(nc: bass.Bass):
    """Keep only the first drain of the end block (it carries the global
    tile-clock sem waits, incl. the output-DMA completion) and drop the two
    all-engine barrier rounds behind it."""
    for blk in nc.m.functions[0].blocks:
        if not blk.name.endswith("_end"):
            continue
        kept, seen_drain = [], False
        for ins in blk.instructions:
            if isinstance(ins, mybir.InstDrain):
                if not seen_drain:
                    kept.append(ins)
                    seen_drain = True
                continue
            if isinstance(ins, mybir.InstEventSemaphore):
                continue
            kept.append(ins)
        blk.instructions = kept


def _col_min_tree(eng, pool, src, k, out_ap, name, dt):
    """Min over the k columns of src ([64, k, B] AP, base partition 0),
    written into out_ap ([64, B], any base partition).  Uses bulk
    first-half-vs-second-half tensor_tensor ops (equal input base
    partitions — required by the BIR verifier)."""
    mn = mybir.AluOpType.min
    cur, i = src, 0
    while k > 3:
        assert k % 2 == 0, f"host must pad col count even, got {k}"
        h = k // 2
        t = pool.tile([64, h, B], dt, tag=f"{name}t{i}")
        eng.tensor_tensor(t[:], cur[:, 0:h, :], cur[:, h:2 * h, :], mn)
        cur, k, i = t[:], h, i + 1
    if k == 3:
        t = pool.tile([64, B], dt, tag=f"{name}p")
        eng.tensor_tensor(t[:], cur[:, 0, :], cur[:, 1, :], mn)
        eng.tensor_tensor(out_ap, t[:], cur[:, 2, :], mn)
    elif k == 2:
        eng.tensor_tensor(out_ap, cur[:, 0, :], cur[:, 1, :], mn)
    else:
        eng.tensor_tensor(out_ap, cur[:, 0, :], cur[:, 0, :], mn)


def _warm_pe(nc, pool, psum, n):
    """Dependency-free dummy matmuls on a scratch tile keep the PE busy
    through the input-DMA wait, so it reaches the full-clock p-state
    (3us of continuous execution) and is still running when the real
    matmuls issue (a gap would reset the ramp).  The memset runs on DVE
    (idle until the first value blob lands)."""
    scr = pool.tile([128, B], mybir.dt.bfloat16, tag="warm_in")
    nc.vector.memset(scr[:], 0.0)
    pscr = psum.tile([128, B], mybir.dt.float32, tag="warm_out")
    for _ in range(n):
        nc.tensor.matmul(pscr[:], scr[:], scr[:], start=True, stop=True)


def _build_p1(KH: int, KL: int, lpos: int, lneg: int) -> bass.Bass:
    """Launch-1 program: f32 end-to-end (u1 and 1-u1 both must stay
    relatively accurate for apply-2's tables)."""
    key = ("p1", KH, KL, lpos, lneg)
    if key in _PROGRAM_CACHE:
        return _PROGRAM_CACHE[key]

    f32 = mybir.dt.float32
    mx, mn = mybir.AluOpType.max, mybir.AluOpType.min
    L = lpos + lneg
    HH = KH // 2 if (KH > 3 and KH % 2 == 0) else KH
    LPAD = 2 * max(lpos, lneg)
    VBW = LPAD * NLOC                    # scat layers (pos | neg, padded)
    nc = bass.Bass(num_devices=NCORES)
    vah1_d = nc.declare_dram_parameter("vah1", [64, HH * B], f32, isOutput=False)
    vah2_d = nc.declare_dram_parameter("vah2", [64, HH * B], f32, isOutput=False)
    val_d = nc.declare_dram_parameter("val", [64, KL * B], f32, isOutput=False)
    vb_d = nc.declare_dram_parameter("vb", [128, VBW], f32, isOutput=False)
    bs_d = nc.declare_dram_parameter("bs", [NLOC, B], f32, isOutput=False)
    u_d = nc.declare_dram_parameter("u", [NLOC, B], f32, isOutput=True)

    with FixedTileContext(nc) as tc:
        with (
            tc.tile_pool(name="sbuf", bufs=1) as pool,
            tc.tile_pool(name="psum", bufs=1, space="PSUM") as psum,
        ):
            _warm_pe(nc, pool, psum, WARM1)
            # DMA plan (consumption/engine order; HWDGE prep is 625ns
            # serialized, the DMA engines pick ready transfers in order):
            #   SP HWDGE : vah1 (first hi half), val (small), vb
            #   Pool SWDGE: vah2 (second hi half), bs (clamp base, last)
            vah1 = pool.tile([64, B, HH], f32)
            nc.sync.dma_start(vah1[:],
                              vah1_d[:].rearrange("p (b k) -> p b k", k=HH))
            vah2 = pool.tile([64, B, HH], f32)
            nc.gpsimd.dma_start(vah2[:],
                                vah2_d[:].rearrange("p (b k) -> p b k", k=HH))
            val = pool.tile([64, KL, B], f32)
            nc.sync.dma_start(val[:],
                              val_d[:].rearrange("p (k b) -> p k b", k=KL))
            vb = pool.tile([128, VBW], f32)
            nc.sync.dma_start(vb[:], vb_d[:])
            bs = pool.tile([NLOC, B], f32)
            nc.gpsimd.dma_start(bs[:], bs_d[:])

            # bm: one bulk tensor_reduce per hi half, min-combined in place;
            # the lo (val) tree is one tt for KL<=2.
            ra = pool.tile([64, B], f32)
            nc.vector.tensor_reduce(out=ra[:], in_=vah1[:],
                                    axis=mybir.AxisListType.X, op=mn)
            bm = pool.tile([128, B], f32)
            nc.vector.tensor_reduce(out=bm[0:64, :], in_=vah2[:],
                                    axis=mybir.AxisListType.X, op=mn)
            nc.vector.tensor_tensor(bm[0:64, :], ra[:], bm[0:64, :], mn)
            _col_min_tree(nc.vector, pool, val[:], KL, bm[64:128, :], "l", f32)

            # f32 matmuls (4 cyc/row) into PAIRED PSUM tiles: left half =
            # neg layer l, right half = pos layer l.  The per-layer +1 (for
            # ub = min_l(1-ps)) rides the const-one bm slot's weight, and the
            # relu floors are absorbed by the median clamp, so the whole
            # post-matmul reduction is one negated min-chain:
            #   t_l = min(t_{l-1}, -P_l)  ->  t = [ub | -lb]
            LP = max(lpos, lneg)
            ps = []
            for l in range(LP):
                pt = psum.tile([NLOC, 2 * B], f32, tag=f"ps{l}")
                nc.tensor.matmul(pt[:, 0:B], vb[:, (LP + l) * NLOC:
                                                 (LP + l + 1) * NLOC],
                                 bm[:], start=True, stop=True)
                nc.tensor.matmul(pt[:, B:2 * B], vb[:, l * NLOC:
                                                    (l + 1) * NLOC],
                                 bm[:], start=True, stop=True)
                ps.append(pt)

            t0 = pool.tile([NLOC, 2 * B], f32, tag="t0")
            nc.vector.tensor_scalar(
                t0[:], ps[0][:], -1.0, None, mybir.AluOpType.mult)
            acc = t0
            for l in range(1, LP):
                nxt = pool.tile([NLOC, 2 * B], f32, tag=f"t{l}")
                nc.vector.scalar_tensor_tensor(
                    nxt[:], ps[l][:], -1.0, acc[:], mybir.AluOpType.mult, mn)
                acc = nxt

            # acc = [ub | -lb]; u = clamp(p, lo, hi) = median(p, lb, ub)
            ubh = acc[:, 0:B]
            nlb = acc[:, B:2 * B]
            lo = pool.tile([NLOC, B], f32)
            nc.vector.scalar_tensor_tensor(
                lo[:], nlb, -1.0, ubh, mybir.AluOpType.mult, mn)
            hi = pool.tile([NLOC, B], f32)
            nc.vector.scalar_tensor_tensor(
                hi[:], nlb, -1.0, ubh, mybir.AluOpType.mult, mx)
            m1 = pool.tile([NLOC, B], f32)
            nc.vector.tensor_tensor(m1[:], bs[:], lo[:], mx)
            u = pool.tile([NLOC, B], f32)
            nc.vector.tensor_tensor(u[:], m1[:], hi[:], mn)
            nc.sync.dma_start(u_d[:], u[:])

    strip_preamble(nc)
    strip_epilogue(nc)
    if SPLIT_WAITS:
        split_multi_waits(nc)
    _PROGRAM_CACHE[key] = nc
    return nc


def _build_p2(KH: int, KL: int, lpos: int, lneg: int) -> bass.Bass:
    """Launch-2 program: bf16, head-sign-specific value spaces (baseline
    structure; changed vs baseline: vb rides the Pool SWDGE ring so the
    matmuls are not gated on its late HWDGE slot, val goes second on SP,
    and the warm count is tuned so the real matmuls hit the full-clock
    p-state with no idle gap)."""
    key = ("p2", KH, KL, lpos, lneg)
    if key in _PROGRAM_CACHE:
        return _PROGRAM_CACHE[key]

    f32, bf16 = mybir.dt.float32, mybir.dt.bfloat16
    mx, mn = mybir.AluOpType.max, mybir.AluOpType.min
    L = lpos + lneg
    VBW = (L + 1) * NLOC + 1             # scat layers | base | zero col
    nc = bass.Bass(num_devices=NCORES)
    HH = KH // 2 if (KH > 3 and KH % 2 == 0) else KH
    vah_d = nc.declare_dram_parameter("vah", [64, HH * B], bf16, isOutput=False)
    vah2_d = None
    if HH != KH:
        vah2_d = nc.declare_dram_parameter("vah2", [64, HH * B], bf16,
                                           isOutput=False)
    val_d = nc.declare_dram_parameter("val", [64, KL * B], bf16, isOutput=False)
    vb_d = nc.declare_dram_parameter("vb", [128, VBW], bf16, isOutput=False)
    u_d = nc.declare_dram_parameter("u", [NLOC, B], bf16, isOutput=True)

    with FixedTileContext(nc) as tc:
        with (
            tc.tile_pool(name="sbuf", bufs=1) as pool,
            tc.tile_pool(name="psum", bufs=1, space="PSUM") as psum,
        ):
            _warm_pe(nc, pool, psum, WARM2)
            vah = pool.tile([64, HH, B], bf16)
            nc.sync.dma_start(vah[:],
                              vah_d[:].rearrange("p (k b) -> p k b", k=HH))
            if vah2_d is not None:
                vah2 = pool.tile([64, HH, B], bf16)
                nc.sync.dma_start(vah2[:],
                                  vah2_d[:].rearrange("p (k b) -> p k b", k=HH))
            val = pool.tile([64, KL, B], bf16)
            nc.gpsimd.dma_start(val[:],
                               val_d[:].rearrange("p (k b) -> p k b", k=KL))
            vb = pool.tile([128, VBW], bf16)
            nc.sync.dma_start(vb[:], vb_d[:])

            bm = pool.tile([128, B], bf16)
            if vah2_d is not None:
                # per-half trees pipeline with the two blob arrivals
                h1 = pool.tile([64, B], bf16, tag="h1")
                _col_min_tree(nc.vector, pool, vah[:], HH, h1[:], "h", bf16)
                h2 = pool.tile([64, B], bf16, tag="h2")
                _col_min_tree(nc.vector, pool, vah2[:], HH, h2[:], "g", bf16)
                nc.vector.tensor_tensor(bm[0:64, :], h1[:], h2[:],
                                        mybir.AluOpType.min)
            else:
                _col_min_tree(nc.vector, pool, vah[:], HH, bm[0:64, :],
                              "h", bf16)
            _col_min_tree(nc.vector, pool, val[:], KL, bm[64:128, :], "l", bf16)

            # neg layers first: the DVE ts-chain consuming them is the
            # post-matmul critical path
            ps = [None] * L
            order = []
            for l in range(max(lpos, lneg)):
                if l < lneg:
                    order.append(lpos + l)
                if l < lpos:
                    order.append(l)
            for l in order:
                pt = psum.tile([NLOC, B], f32, tag=f"ps{l}")
                nc.tensor.matmul(pt[:], vb[:, l * NLOC:(l + 1) * NLOC],
                                 bm[:], start=True, stop=True)
                ps[l] = pt

            # lb = max_l relu(ps_l): relu of the first layer on Act, the
            # rest folded into scalar_tensor_tensor max steps on DVE
            zbias = vb[:, VBW - 1:VBW]
            r0 = pool.tile([NLOC, B], bf16, tag="r0")
            nc.scalar.activation(
                r0[:], ps[0][:], mybir.ActivationFunctionType.Relu, bias=zbias)
            lb = r0
            for l in range(1, lpos):
                nxt = pool.tile([NLOC, B], bf16, tag=f"lb{l}")
                nc.vector.scalar_tensor_tensor(
                    nxt[:], ps[l][:], 0.0, lb[:], mx, mx)
                lb = nxt

            # ub = min_l(-ps_l): the neg-layer static bias (+2 on empty
            # cells) rides the const-one slot's scatter weight, so the side
            # is a pure negated min-chain: Act copy first, stt steps after.
            s0 = pool.tile([NLOC, B], bf16, tag="s0")
            nc.scalar.activation(
                s0[:], ps[lpos][:], mybir.ActivationFunctionType.Copy,
                bias=0.0, scale=-1.0)  # Copy takes float bias as immediate
            ub = s0
            for l in range(1, lneg):
                nxt = pool.tile([NLOC, B], bf16, tag=f"ub{l}")
                nc.vector.scalar_tensor_tensor(
                    nxt[:], ps[lpos + l][:], -1.0, ub[:],
                    mybir.AluOpType.mult, mn)
                ub = nxt

            # u = clamp(base, lo, hi) = min(max(base, lo), hi)
            base = vb[:, L * NLOC:(L + 1) * NLOC]
            lo = pool.tile([NLOC, B], bf16)
            nc.vector.tensor_tensor(lo[:], lb[:], ub[:], mn)
            hi = pool.tile([NLOC, B], bf16)
            nc.vector.tensor_tensor(hi[:], lb[:], ub[:], mx)
            m1 = pool.tile([NLOC, B], bf16)
            nc.vector.tensor_tensor(m1[:], base, lo[:], mx)
            u = pool.tile([NLOC, B], bf16)
            nc.vector.tensor_tensor(u[:], m1[:], hi[:], mn)
            nc.sync.dma_start(u_d[:], u[:])

    strip_preamble(nc)
    strip_epilogue(nc)
    if SPLIT_WAITS:
        split_multi_waits(nc)
    _PROGRAM_CACHE[key] = nc
    return nc


def _pad_k(k):
    """Smallest col count >= k that the bulk halving tree accepts
    (k = m * 2^j with m in {1,2,3})."""
    if k <= 3:
        return k
    c = 4
    while True:
        for m in (4, 6):
            if m * c // 4 >= k:
                return m * c // 4
        c *= 2


def _pack_rows(row_lists_core, KH, KL):
    """hi slots (0:64) -> rows_hi[s] cols 0..KH; lo slots (64:128) ->
    rows_lo[s-64] cols 0..KL; ZROW pads."""
    rows_hi = np.full((64, KH), ZROW, dtype=np.int64)
    rows_lo = np.full((64, KL), ZROW, dtype=np.int64)
    for s, rr in enumerate(row_lists_core):
        if s < 64:
            rows_hi[s, :len(rr)] = rr
        else:
            rows_lo[s - 64, :len(rr)] = rr
    return rows_hi, rows_lo


class _Prep:
    """Host-side, launch-independent preprocessing (slot assignment, literal
    row ids, scatter one-hots)."""

    def __init__(self, preds, goal, atoms, pos_body, neg_body, pos_head, neg_head):
        f32 = np.float32
        import ml_dtypes
        self.bf16 = ml_dtypes.bfloat16
        self.atoms = np.asarray(atoms)
        self.p = preds[:, self.atoms].astype(f32)            # [B, NA]
        self.g = goal[:, self.atoms].astype(f32)
        self.pT = np.ascontiguousarray(self.p.T)             # [NA, B]
        self.gT = np.ascontiguousarray(self.g.T)

        hsum = pos_head + neg_head
        assert np.all(hsum.sum(axis=1) == 1.0), "heads must be one-hot"
        self.h = np.argmax(hsum, axis=1)                     # [C]
        self.head_is_pos = pos_head[np.arange(C), self.h] == 1.0
        owner = self.h // NLOC

        pos_lists = [np.nonzero(pos_body[c])[0] for c in range(C)]
        neg_lists = [np.nonzero(neg_body[c])[0] for c in range(C)]
        # launch-1 rows: complement space for every slot, no head rows
        row1 = [np.concatenate([REG_VP + pos_lists[c], REG_VN + neg_lists[c]])
                .astype(np.int64) for c in range(C)]
        # launch-2 rows: sign-specific space + head-activation row
        row2 = []
        for c in range(C):
            if self.head_is_pos[c]:
                rp, rn, rh = REG_VP, REG_VN, REG_HP
            else:
                rp, rn, rh = REG_WP, REG_WN, REG_HN
            row2.append(np.concatenate([
                rp + pos_lists[c], rn + neg_lists[c],
                [rh + self.h[c]]]).astype(np.int64))
        ncnt = np.array([len(r) for r in row2])

        self.cons = []
        k1h = k1l = k2h = k2l = 1
        for i in range(NCORES):
            ci = np.nonzero(owner == i)[0]
            assert len(ci) <= SLOTS, f"core {i} has {len(ci)} constraints"
            ci = ci[np.argsort(-ncnt[ci], kind="stable")]
            self.cons.append(ci)
            c2 = ncnt[ci]
            k2h = max(k2h, int(c2[:64].max(initial=0)))
            k2l = max(k2l, int(c2[64:].max(initial=0)))
            c1 = c2 - 1                                      # no head row
            k1h = max(k1h, int(c1[:64].max(initial=0)))
            k1l = max(k1l, int(c1[64:].max(initial=0)))
        self.key1 = (_pad_k(k1h), _pad_k(k1l))
        self.key2 = (_pad_k(k2h), _pad_k(k2l))

        self.rows1 = []
        self.rows2 = []
        lpos_need = lneg_need = 1
        layer_asn = []
        for i in range(NCORES):
            ci = self.cons[i]
            self.rows1.append(_pack_rows([row1[c] for c in ci], *self.key1))
            self.rows2.append(_pack_rows([row2[c] for c in ci], *self.key2))

            counts = {}
            asn = []
            for s, c in enumerate(ci):
                key = (self.h[c] % NLOC, bool(self.head_is_pos[c]))
                l = counts.get(key, 0)
                counts[key] = l + 1
                asn.append((s, key[1], l, key[0]))
                if key[1]:
                    lpos_need = max(lpos_need, l + 1)
                else:
                    lneg_need = max(lneg_need, l + 1)
            layer_asn.append(asn)

        self.lpos, self.lneg = lpos_need, lneg_need
        LP = max(self.lpos, self.lneg)
        self.LP = LP
        # Scatter one-hot layers, [pos 0..LP | neg 0..LP].  Slot CONST_SLOT
        # (guaranteed free: <=127 constraints per core) has bm == 1 (all-pad
        # rows); its weight carries the per-layer bias: -1 on launch-1 neg
        # layers (ub = min_l(1-ps)), -(0|2) empty-cell bias on launch-2 neg
        # layers.  Relu floors are absorbed by the median clamp, so pos
        # layers need no bias at all.
        L = self.lpos + self.lneg
        self.scat1 = []       # p1: [128, 2*LP*NLOC] f32, const-slot biases
        self.scat2 = []       # p2: [128, L*NLOC] one-hots + const-slot bias
        for i in range(NCORES):
            assert len(self.cons[i]) <= SLOTS - 1, "need a free const slot"
            sc1 = np.zeros((SLOTS, 2 * LP, NLOC), dtype=f32)
            sc2 = np.zeros((SLOTS, L, NLOC), dtype=f32)
            occ = np.zeros((max(self.lneg, 1), NLOC), dtype=bool)
            for s, is_pos, l, n in layer_asn[i]:
                sc1[s, l if is_pos else LP + l, n] = 1.0
                sc2[s, l if is_pos else self.lpos + l, n] = 1.0
                if not is_pos:
                    occ[l, n] = True
            sc1[CONST_SLOT, LP:, :] = -1.0
            for l in range(self.lneg):
                sc2[CONST_SLOT, self.lpos + l, :] = np.where(
                    occ[l], 0.0, -2.0)
            self.scat1.append(np.ascontiguousarray(
                sc1.reshape(SLOTS, 2 * LP * NLOC)))
            self.scat2.append(np.ascontiguousarray(
                sc2.reshape(SLOTS, L * NLOC)).astype(self.bf16))

    def gather1(self, vcat):
        """Launch-1 f32 blobs: hi halves b-major [64, HH*B], val k-major."""
        KH, KL = self.key1
        HH = KH // 2 if (KH > 3 and KH % 2 == 0) else KH
        f32 = np.float32
        out = []
        for i in range(NCORES):
            rh, rl = self.rows1[i]
            out.append({
                "vah1": np.ascontiguousarray(
                    vcat[rh[:, :HH]].transpose(0, 2, 1)
                    .reshape(64, HH * B)).astype(f32),
                "vah2": np.ascontiguousarray(
                    vcat[rh[:, HH:2 * HH]].transpose(0, 2, 1)
                    .reshape(64, HH * B)).astype(f32),
                "val": np.ascontiguousarray(
                    vcat[rl].reshape(64, KL * B)).astype(f32),
            })
        return out

    def gather2(self, vcat):
        """Launch-2 bf16 blobs: vah k-major in two halves, val k-major."""
        KH, KL = self.key2
        HH = KH // 2 if (KH > 3 and KH % 2 == 0) else KH
        out = []
        for i in range(NCORES):
            rh, rl = self.rows2[i]
            m = {
                "vah": np.ascontiguousarray(
                    vcat[rh[:, :HH]].reshape(64, HH * B)).astype(self.bf16),
                "val": np.ascontiguousarray(
                    vcat[rl].reshape(64, KL * B)).astype(self.bf16),
            }
            if HH != KH:
                m["vah2"] = np.ascontiguousarray(
                    vcat[rh[:, HH:2 * HH]].reshape(64, HH * B)
                ).astype(self.bf16)
            out.append(m)
        return out


def kernel(preds, goal, atoms, pos_body, neg_body, pos_head, neg_head):
    preds = np.asarray(preds)
    f32 = np.float32
    prep = _Prep(np.asarray(preds, f32), np.asarray(goal, f32),
                 atoms, np.asarray(pos_body, f32),
                 np.asarray(neg_body, f32),
                 np.asarray(pos_head, f32),
                 np.asarray(neg_head, f32))
    nc1 = _build_p1(*prep.key1, prep.lpos, prep.lneg)
    nc2 = _build_p2(*prep.key2, prep.lpos, prep.lneg)
    core_ids = list(range(NCORES))
    g, p = prep.gT, prep.pT

    # ---- launch 1 (f32): complement-space table, goal-folded actives ----
    vcat1 = np.concatenate([
        np.where(g == 1.0, p, -1.0),             # v'+ = 1 - (1-p), sat-gated
        np.where(g == 0.0, 1.0 - p, -1.0),       # v'- = 1 - p-lit
        np.ones((4 * NA + 1, B), f32),           # unused regions + pad row
    ], axis=0)
    vas = prep.gather1(vcat1)
    in_maps = [dict(vas[i], vb=prep.scat1[i],
                    bs=np.ascontiguousarray(p[i * NLOC:(i + 1) * NLOC]))
               for i in range(NCORES)]
    res = run_bass_kernel_spmd(nc1, in_maps, core_ids)
    u1T = np.concatenate(
        [res.results[i]["u"].astype(f32) for i in range(NCORES)], axis=0)

    # ---- launch 2 (bf16): sign-specific spaces + head-activation rows ----
    a_ = (1.0 - g) * (1.0 - u1T)
    b_ = g * u1T
    vcat2 = np.concatenate([
        1.0 - a_,                                # v'+   (pos-head slots)
        1.0 - b_,                                # v'-
        -a_,                                     # -v+   (neg-head slots)
        -b_,                                     # -v-
        1.0 - 2.0 * g,                           # head-act, pos slots
        2.0 * g - 2.0,                           # head-act, neg slots
        np.ones((1, B), f32),                    # pad row
    ], axis=0)
    vas = prep.gather2(vcat2)
    zcol = np.zeros((128, 1), prep.bf16)
    in_maps = []
    for i in range(NCORES):
        vb = np.concatenate([
            prep.scat2[i],
            np.ascontiguousarray(
                u1T[i * NLOC:(i + 1) * NLOC]).astype(prep.bf16),
            zcol,
        ], axis=1)
        in_maps.append(dict(vas[i], vb=vb))
    res = run_bass_kernel_spmd(nc2, in_maps, core_ids)
    u2T = np.concatenate(
        [res.results[i]["u"].astype(f32) for i in range(NCORES)], axis=0)

    out = np.array(preds, dtype=preds.dtype, copy=True)
    out[:, prep.atoms] = u2T.T.astype(preds.dtype)
    return out
